# revision 1
# baseline (speedup 1.0000x reference)
"""DCN layer kernel for Trainium2 (raw Bass), 8-core data parallel.

Computes out = x_0 * (x_l @ w) + b[:, 0] + x_l for
x_l, x_0: [65536, 1024] f32, w, b: [1024, 1] f32.

Sharding: batch dim split evenly across 8 NeuronCores; w/b replicated.

Host side interleaves x_l/x_0 row blocks into one tensor and
pre-replicates w across the 128 partitions. Per core, a 3-stage
software pipeline over tiles of K=8 row blocks (8MB loads):
  SP   (HWDGE): load interleaved x tiles into a 2-slot SBUF ring
  DVE         : 4 batched ops per tile over [128, K, 1024]:
                  tmp = x_l * w_bcast        (free-dim stride-0 bcast)
                  s[P,K] = rowsum(tmp)       (innermost-axis reduce)
                  m = x_0 * s_bcast          (stride-0 bcast of s)
                  o = m + x_l                (written over the x_0 half)
  ACT  (HWDGE): store finished o tiles
Raw Bass with standalone wait_ge commands — every instruction carries at
most one semaphore wait (this toolchain's walrus rejects more). Each ring
slot has its own DMA-completion semaphore so at most one DMA is in flight
per semaphore (concurrent 16-way SDMA increments on a shared semaphore
would make thresholds ambiguous). Same-engine DVE RAW needs an explicit
chain semaphore (verified on HW: without it, reads race ahead of writes).
"""

from contextlib import ExitStack

import numpy as np

import concourse.bass as bass
from concourse import mybir
from concourse import bass_utils

P = 128  # SBUF partitions
N_CORES = 8
K = 8  # row blocks per tile (8MB x-tile)
XB = 2  # x ring slots

f32 = mybir.dt.float32
MUL = mybir.AluOpType.mult
ADD = mybir.AluOpType.add


def _build(nb, dim, with_b, repeat=1):
    """Per-core program: nb 128-row blocks of width dim, K blocks per tile."""
    assert nb % K == 0
    nt = nb // K
    nit = nt * repeat  # repeat>1 re-runs the pipeline for wall-clock timing
    nc = bass.Bass("TRN2", target_bir_lowering=False, debug=False,
                   enable_asserts=False)
    xin = nc.dram_tensor("xin", [nb, P, 2, dim], f32, kind="ExternalInput").ap()
    w_rep_d = nc.dram_tensor("w_rep_in", [P, dim], f32, kind="ExternalInput").ap()
    if with_b:
        b_rep_d = nc.dram_tensor("b_rep_in", [P, dim], f32, kind="ExternalInput").ap()
    out = nc.dram_tensor("out", [nb * P, dim], f32, kind="ExternalOutput").ap()

    xin_t = xin.rearrange("(t k) p c d -> t p k c d", k=K)  # [nt, P, K, 2, dim]
    out_t = out.rearrange("(t k p) d -> t p k d", p=P, k=K)  # [nt, P, K, dim]

    n_const = 1 + int(with_b)

    with ExitStack() as ctx:
        e = ctx.enter_context
        xbuf = e(nc.sbuf_tensor([P, XB, K, 2, dim], f32))
        tmp = e(nc.sbuf_tensor([P, K, dim], f32))
        wrep = e(nc.sbuf_tensor([P, dim], f32))
        brep = e(nc.sbuf_tensor([P, dim], f32))
        s = e(nc.sbuf_tensor([P, K], f32))
        const_sem = e(nc.semaphore("const_sem"))
        load_sems = [e(nc.semaphore(f"load_sem{j}")) for j in range(XB)]
        store_sems = [e(nc.semaphore(f"store_sem{j}")) for j in range(XB)]
        dve_sem = e(nc.semaphore("dve_sem"))
        chain_sem = e(nc.semaphore("chain_sem"))
        block = e(nc.Block())

        @block.sync
        def _(sync):
            sync.dma_start(out=wrep[:, :], in_=w_rep_d[:, :]).then_inc(const_sem, 16)
            if with_b:
                sync.dma_start(out=brep[:, :], in_=b_rep_d[:, :]).then_inc(
                    const_sem, 16
                )
            for t in range(nit):
                if t >= XB:
                    # slot free only after its previous store (o lives in the
                    # x_0 half of the slot) fully landed in DRAM
                    sync.wait_ge(store_sems[t % XB], 16 * (t // XB))
                sync.dma_start(
                    out=xbuf[:, t % XB, :, :, :], in_=xin_t[t % nt]
                ).then_inc(load_sems[t % XB], 16)

        @block.vector
        def _(vector):
            cnt = [0]

            def chain(inst):
                inst.then_inc(chain_sem, 1)
                cnt[0] += 1
                vector.wait_ge(chain_sem, cnt[0])
                return inst

            vector.wait_ge(const_sem, 16 * n_const)
            w_b = wrep[:, None, :].broadcast_to([P, K, dim])
            if with_b:
                b_b = brep[:, None, :].broadcast_to([P, K, dim])
            s_b = s[:, :, None].broadcast_to([P, K, dim])
            for t in range(nit):
                sl = t % XB
                vector.wait_ge(load_sems[sl], 16 * (t // XB + 1))
                xl = xbuf[:, sl, :, 0, :]  # [P, K, dim]
                x0 = xbuf[:, sl, :, 1, :]  # [P, K, dim]; overwritten by o
                # tmp = x_l * w
                chain(nc.vector.scalar_tensor_tensor(
                    out=tmp[:, :, :], in0=xl, scalar=1.0, in1=w_b,
                    op0=MUL, op1=MUL,
                ))
                # s[p, k] = sum_d tmp[p, k, d]
                chain(nc.vector.tensor_reduce(
                    s[:, :], tmp[:, :, :], axis=mybir.AxisListType.X, op=ADD
                ))
                # m = x_0 * s  (reuses tmp)
                chain(nc.vector.scalar_tensor_tensor(
                    out=tmp[:, :, :], in0=x0, scalar=1.0, in1=s_b,
                    op0=MUL, op1=MUL,
                ))
                # o = m + x_l (+ b), written over the x_0 half of the slot
                if with_b:
                    chain(nc.vector.scalar_tensor_tensor(
                        out=x0, in0=tmp[:, :, :], scalar=0.0, in1=xl,
                        op0=ADD, op1=ADD,
                    ))
                    last = nc.vector.scalar_tensor_tensor(
                        out=x0, in0=x0, scalar=0.0, in1=b_b, op0=ADD, op1=ADD
                    )
                else:
                    last = nc.vector.scalar_tensor_tensor(
                        out=x0, in0=tmp[:, :, :], scalar=0.0, in1=xl,
                        op0=ADD, op1=ADD,
                    )
                last.then_inc(dve_sem, 1)

        @block.scalar
        def _(scalar):
            for t in range(nit):
                scalar.wait_ge(dve_sem, t + 1)
                scalar.dma_start(
                    out=out_t[t % nt], in_=xbuf[:, t % XB, :, 1, :]
                ).then_inc(store_sems[t % XB], 16)
            # drain: all stores landed before program end
            for j in range(XB):
                n_j = (nit - 1 - j) // XB + 1 if j < nit else 0
                if n_j:
                    scalar.wait_ge(store_sems[j], 16 * n_j)

    return nc


_cache = {}


def _get_module(nb, dim, with_b, repeat=1):
    key = (nb, dim, with_b, repeat)
    if key not in _cache:
        _cache[key] = _build(nb, dim, with_b, repeat)
    return _cache[key]


def make_inputs(x_l, x_0, w, b, n_cores=N_CORES):
    """Host-side shard + interleave. Returns (in_maps, with_b, nb, dim)."""
    rows, dim = x_l.shape
    assert rows % (n_cores * P) == 0
    bl = rows // n_cores
    nb = bl // P
    with_b = bool(np.any(b))
    xin = np.stack([x_l, x_0], axis=1)  # [rows, 2, dim]
    w_rep = np.ascontiguousarray(np.broadcast_to(w.reshape(1, dim), (P, dim)))
    if with_b:
        b_rep = np.ascontiguousarray(np.broadcast_to(b.reshape(1, dim), (P, dim)))
    in_maps = []
    for i in range(n_cores):
        m = {
            "xin": xin[i * bl : (i + 1) * bl].reshape(nb, P, 2, dim),
            "w_rep_in": w_rep,
        }
        if with_b:
            m["b_rep_in"] = b_rep
        in_maps.append(m)
    return in_maps, with_b, nb, dim


def run_sharded(x_l, x_0, w, b, trace=False, repeat=1, **kw):
    in_maps, with_b, nb, dim = make_inputs(x_l, x_0, w, b)
    nc = _get_module(nb, dim, with_b, repeat=repeat)
    res = bass_utils.run_bass_kernel_spmd(
        nc, in_maps, core_ids=list(range(N_CORES)), trace=trace, **kw
    )
    out = np.concatenate([res.results[i]["out"] for i in range(N_CORES)], axis=0)
    return out, res


def kernel(x_l, x_0, w, b):
    out, _ = run_sharded(
        np.asarray(x_l), np.asarray(x_0), np.asarray(w), np.asarray(b)
    )
    return out.astype(np.float32, copy=False)

